# revision 53
# baseline (speedup 1.0000x reference)
"""Trainium2 Bass kernel for AspectNeighborAttention (gnn_message_passing).

Pure data-parallel over batch: 32 batches -> 8 NeuronCores x 4 batches.
All weights replicated, host-converted to bf16 and host-PRE-TRANSPOSED into
the chunk-major [128, KC, *] lhsT/rhs layouts the TensorEngine wants, so the
device does plain contiguous DMAs only. dep is host-bf16 (halves HBM traffic).

Key structure (derived over several profiled iterations; 290us -> ~166us):
  * W-folding kills the nbr intermediate entirely:
      temp = attn @ (zs @ G0^T) + D @ G1 + zs @ WhZ^T - bertS
    with G0 = WhN @ WfZ and G1 = (WhN @ WfE)^T precomputed on host, and
    -bertS folded into the same PSUM accumulation via bertsT x (-I) chunk
    matmuls, so the blend is a single DVE scalar_tensor_tensor reading
    PSUM: out = upd * psum + bertS (upd = host-folded span & any-neighbor
    mask, broadcast to a PSUM column by a rank-1 PE matmul).
  * All but 2 small temp matmuls pre-accumulate before the D reduction
    finishes, so the post-softmax serial tail is ~2us.
  * wa_e is folded into the host f32->bf16 cast of dep (depW = dep*wa_e)
    and 1/wa_e into G1, so s_e = reduce_e(depW) is a single DVE
    tensor_reduce (axis=X is its hard 1x floor) with NO multiply, and
    D' @ G1' is exactly D @ G1 (the diagonal rescale cancels; zeros in
    wa_e are guarded). Pool gets NO multiplies: measured Pool
    TENSOR_TENSOR MULTIPLY carries a ~5-8us fixed overhead regardless of
    size (ADDs are fine at ~1-2 ns/elem).
  * D = reduce_j(attn * dep) in natural [i,j,e] layout: DVE mult in two
    j-halves (so the Pool tree can start after the first half), then a
    pairwise j-tree ping-ponged between two scratch tiles, split DVE
    rows [64:128) / Pool rows [0:64).
  * lrelu on DVE as max(0.01x, x); ACT keeps only Exp (no table thrash)
    plus PSUM->SBUF copies; softmax masking via additive C-shift, and
    attn off-neighbor entries are exact 0 through exp underflow (no mask
    multiply needed).
  * Engine-queue ordering matters (in-order queues): emission order is
    queue order; deferring ops that wait on long cross-engine chains
    avoids head-of-line blocking, but holding the D-tree ping-pong tiles
    across a batch boundary (fully-serial Pool tree + deferred blend)
    stalls the next batch's D-mult on the tile ring - keep the tree split
    and the blend inline.
The remaining wall is the Vector engine (~70% busy, <6us idle) plus a
hardware power throttle (~50% util cap for ~2/3 of the kernel, visible in
the profile summary; it also makes single-run A/B deltas under ~25us
unresolvable); se-reduce and the D-mult are at their measured floors.

The roll(z,-1)/roll(out,+1) pair is handled purely with shifted-row DMAs
(bert is sent pre-rolled as `berts`).
"""

import sys

for _p in ("/opt/trn_rl_repo",):
    if _p not in sys.path:
        sys.path.insert(0, _p)

import os
import numpy as np
import ml_dtypes

import concourse.bass as bass
import concourse.bacc as bacc_mod
import concourse.mybir as mybir
import concourse.tile as tile
from concourse.masks import make_identity

B, L, H, E = 32, 128, 768, 64
NCORES = 8
PB = B // NCORES  # batches per core
KC = H // 128     # 6 k-chunks
F32 = mybir.dt.float32
BF16 = mybir.dt.bfloat16
AF = mybir.ActivationFunctionType
OP = mybir.AluOpType
AX = mybir.AxisListType
MASK_SHIFT = 10000.0  # additive mask offset (see score masking)

_CACHED = {}

CFG = dict(
    dep_bufs=int(os.environ.get("K_DEP_BUFS", 3)),
    ttmp_bufs=int(os.environ.get("K_TTMP_BUFS", 3)),
    spool_bufs=int(os.environ.get("K_SPOOL_BUFS", 3)),
    opool_bufs=int(os.environ.get("K_OPOOL_BUFS", 3)),
    ptr_bufs=int(os.environ.get("K_PTR_BUFS", 3)),
    pbig_bufs=int(os.environ.get("K_PBIG_BUFS", 2)),
    jpd=int(os.environ.get("K_JPD", 20)),  # D-mult j-split: [0,jpd) Pool
    jp1=int(os.environ.get("K_JP1", 0)),  # s_e-mult j-split: [0,jp1) Pool
)


def _build(debug=False):
    nc = bacc_mod.Bacc("TRN2", target_bir_lowering=False, debug=False,
                       num_devices=NCORES)

    bert = nc.dram_tensor("berts", [PB, L, H], F32, kind="ExternalInput")
    bertsT = nc.dram_tensor("bertsT", [PB, 128, KC, 128], BF16,
                            kind="ExternalInput")
    dep = nc.dram_tensor("dep", [PB, L, L, E], BF16, kind="ExternalInput")
    adjf = nc.dram_tensor("adjf", [PB, L, L], F32, kind="ExternalInput")
    vrow = nc.dram_tensor("vrow", [1, PB, 128], BF16, kind="ExternalInput")
    wzT_d = nc.dram_tensor("wzT", [128, KC, H], BF16, kind="ExternalInput")
    g0T_d = nc.dram_tensor("g0T", [128, KC, H], BF16, kind="ExternalInput")
    whzT_d = nc.dram_tensor("whzT", [128, KC, H], BF16, kind="ExternalInput")
    g1_d = nc.dram_tensor("g1", [E, H], BF16, kind="ExternalInput")
    w2T_d = nc.dram_tensor("w2T", [128, KC, 2], BF16, kind="ExternalInput")
    bzt = nc.dram_tensor("bzt", [1, H], BF16, kind="ExternalInput")
    bat = nc.dram_tensor("bat", [1, 1], F32, kind="ExternalInput")
    out = nc.dram_tensor("out", [PB, L, H], F32, kind="ExternalOutput")

    dbg = {}
    if debug:
        for nm, shape, dt in [
            ("d_zsT", [128, KC, 128], BF16), ("d_si", [1, 128], F32),
            ("d_sjb", [1, 128], F32), ("d_se", [128, L], BF16),
            ("d_masked", [128, L], F32), ("d_attn", [128, L], BF16),
            ("d_dvec", [128, E], BF16), ("d_ab", [128, H], BF16),
            ("d_nbrT", [128, KC, 128], BF16), ("d_tempb", [128, H], F32),
            ("d_upd", [128, 1], F32), ("d_scb", [128, 128], F32),
        ]:
            dbg[nm] = nc.dram_tensor(nm, shape, dt, kind="ExternalOutput")
    with tile.TileContext(nc) as tc:
        with nc.allow_low_precision("bf16 softmax/D path, 2e-2 rel-err gate"):
            _body(tc, nc, bert, bertsT, dep, adjf, vrow, wzT_d, g0T_d,
                  whzT_d, g1_d, w2T_d, bzt, bat, out, dbg)
    nc.compile()
    return nc


def _body(tc, nc, bert, bertsT, dep, adjf, vrow, wzT_d, g0T_d,
          whzT_d, g1_d, w2T_d, bzt, bat, out, dbg=None):
    def dump(name, ap):
        if dbg and name in dbg:
            nc.sync.dma_start(dbg[name][...], ap)
    import contextlib
    cfg = CFG
    JPD = cfg["jpd"]
    JP1 = cfg["jp1"]
    ctx = contextlib.ExitStack()
    with ctx:
        wpool = ctx.enter_context(tc.tile_pool(name="weights", bufs=1))
        dpool = ctx.enter_context(
            tc.tile_pool(name="dep", bufs=cfg["dep_bufs"]))
        tpool = ctx.enter_context(
            tc.tile_pool(name="ttmp", bufs=cfg["ttmp_bufs"]))
        spool = ctx.enter_context(
            tc.tile_pool(name="small", bufs=cfg["spool_bufs"]))
        opool = ctx.enter_context(
            tc.tile_pool(name="outp", bufs=cfg["opool_bufs"]))
        p_tr = ctx.enter_context(
            tc.tile_pool(name="p_tr", bufs=cfg["ptr_bufs"], space="PSUM"))
        p_big = ctx.enter_context(
            tc.tile_pool(name="p_big", bufs=cfg["pbig_bufs"], space="PSUM"))

        # ---------------- one-time setup (plain DMAs only) ----------------
        wzT = wpool.tile([128, KC, H], BF16, tag="wzT")
        nc.sync.dma_start(wzT[:], wzT_d[...])
        g0T = wpool.tile([128, KC, H], BF16, tag="g0T")
        nc.sync.dma_start(g0T[:], g0T_d[...])
        whzT = wpool.tile([128, KC, H], BF16, tag="whzT")
        nc.sync.dma_start(whzT[:], whzT_d[...])
        g1 = wpool.tile([E, H], BF16, tag="g1")
        nc.sync.dma_start(g1[:], g1_d[...])
        w2T = wpool.tile([128, KC, 2], BF16, tag="w2T")
        nc.sync.dma_start(w2T[:], w2T_d[...])
        bzr = wpool.tile([1, H], BF16, tag="bzr")
        nc.sync.dma_start(bzr[:], bzt[:, :])
        bar = wpool.tile([1, 1], F32, tag="bar")
        nc.sync.dma_start(bar[:], bat[:, :])
        vrow4 = wpool.tile([1, PB, 128], BF16, tag="vrow4")
        nc.sync.dma_start(vrow4[:], vrow[:, :, :])

        ones_f = wpool.tile([1, 128], F32, tag="ones_f")
        nc.gpsimd.memset(ones_f[:], 1.0)
        ones_b = wpool.tile([1, 128], BF16, tag="ones_b")
        nc.gpsimd.memset(ones_b[:], 1.0)
        id_bf = wpool.tile([128, 128], BF16, tag="id_bf")
        make_identity(nc, id_bf[:])
        id_negb = wpool.tile([128, 128], BF16, tag="id_negb")
        nc.vector.tensor_scalar(id_negb[:], id_bf[:], -1.0, None, op0=OP.mult)

        # -------- per-batch pipeline, software-pipelined --------
        # The blend/store of batch b-1 is emitted mid-iteration-b so it
        # never head-of-line-blocks the next batch's front-end work on the
        # in-order engine queues.
        def emit_front(b):
            st = {}
            # bertS: rows shifted by one token (z roll); f32 exact for blend
            bertS = spool.tile([128, H], F32, tag="bertS")
            nc.sync.dma_start(bertS[:], bert[b, :, :])
            bertST = spool.tile([128, KC, 128], BF16, tag="bertST")
            nc.sync.dma_start(bertST[:], bertsT[b, :, :, :])
            st["bertST"] = bertST
            dept = dpool.tile([128, L, E], BF16, tag="dept")
            nc.sync.dma_start(dept[:], dep[b, :, :, :])
            adjt = spool.tile([128, L], F32, tag="adjt")
            nc.sync.dma_start(adjt[:], adjf[b, :, :])
            st.update(bertS=bertS, dept=dept, adjt=adjt)

            # ---- zs^T = Wz @ bertS^T + bz ----
            p_z = p_big.tile([128, H], F32, tag="p_big")
            for hc in range(KC):
                ns = slice(hc * 128, (hc + 1) * 128)
                for kc in range(KC):
                    nc.tensor.matmul(p_z[:, ns], wzT[:, kc, ns],
                                     bertST[:, kc, :],
                                     start=(kc == 0), stop=False)
                nc.tensor.matmul(p_z[:, ns], bzr[0:1, ns], ones_b[:],
                                 start=False, stop=True)
            zsT = spool.tile([128, KC, 128], BF16, tag="zsT")
            nc.scalar.copy(zsT[:], p_z[:])
            if b == 0:
                dump("d_zsT", zsT[:])

            # ---- s_i col, (s_j + ba) row-bcast score base ----
            p_s3 = p_tr.tile([128, 512], F32, tag="p_tr")
            for kc in range(KC):
                nc.tensor.matmul(p_s3[0:1, 0:128], w2T[:, kc, 0:1],
                                 zsT[:, kc, :],
                                 start=(kc == 0), stop=(kc == KC - 1))
            for kc in range(KC):
                nc.tensor.matmul(p_s3[0:1, 128:256], w2T[:, kc, 1:2],
                                 zsT[:, kc, :],
                                 start=(kc == 0), stop=(kc == KC - 1))
            si_row = spool.tile([1, 128], F32, tag="si_row")
            nc.scalar.copy(si_row[:], p_s3[0:1, 0:128])
            sjb = spool.tile([1, 128], F32, tag="sjb")
            nc.vector.tensor_scalar(sjb[:], p_s3[0:1, 128:256], bar[0:1, 0:1],
                                    None, op0=OP.add)
            nc.tensor.matmul(p_s3[:, 384:385], si_row[:], ones_f[0:1, 0:1],
                             start=True, stop=True)
            nc.tensor.matmul(p_s3[:, 256:384], ones_f[:], sjb[:],
                             start=True, stop=True)
            if b == 0:
                dump("d_si", si_row[:])
                dump("d_sjb", sjb[:])

            # ---- s_e = reduce_e(depW): wa_e is host-folded into dep,
            # and 1/wa_e into G1, so no multiply is needed here at all ----
            se = spool.tile([128, L], BF16, tag="se")
            nc.vector.tensor_reduce(se[:], dept[:], axis=AX.X, op=OP.add)
            if b == 0:
                dump("d_se", se[:])

            # ---- score = lrelu(se + si + sj + ba); masked; softmax ----
            sadd = spool.tile([128, L], F32, tag="sadd")
            nc.vector.scalar_tensor_tensor(
                sadd[:], se[:], p_s3[:, 384:385], p_s3[:, 256:384],
                op0=OP.add, op1=OP.add)
            score = spool.tile([128, L], F32, tag="score")
            nc.vector.scalar_tensor_tensor(
                score[:], sadd[:], 0.01, sadd[:], op0=OP.mult, op1=OP.max)
            masked = spool.tile([128, L], F32, tag="masked")
            nc.vector.scalar_tensor_tensor(
                masked[:], score[:], MASK_SHIFT, adjt[:],
                op0=OP.add, op1=OP.mult)
            if b == 0:
                dump("d_masked", masked[:])
            mxn = spool.tile([128, 1], F32, tag="mxn")
            nc.vector.tensor_reduce(mxn[:], masked[:], axis=AX.X, op=OP.max,
                                    negate=True)
            ex = spool.tile([128, L], F32, tag="ex")
            sumex = spool.tile([128, 1], F32, tag="sumex")
            nc.scalar.activation(ex[:], masked[:], AF.Exp, bias=mxn[:],
                                 scale=1.0, accum_out=sumex[:])
            rec = spool.tile([128, 1], F32, tag="rec")
            nc.vector.reciprocal(rec[:], sumex[:])
            attnb = spool.tile([128, L], BF16, tag="attnb")
            nc.vector.tensor_scalar(attnb[:], ex[:], rec[0:128, 0:1], None,
                                    op0=OP.mult)
            if b == 0:
                dump("d_attn", attnb[:])

            # attn^T transpose early (needs only attnb)
            p_ad = p_tr.tile([128, 256], BF16, tag="p_tr")
            nc.tensor.transpose(p_ad[:, 0:128], attnb[:], id_bf[:])
            attnT = spool.tile([128, 128], BF16, tag="attnT")
            nc.scalar.copy(attnT[:], p_ad[:, 0:128])
            st.update(zsT=zsT, attnb=attnb, p_ad=p_ad, attnT=attnT)
            return st

        def emit_back(b, st):
            zsT, dept, attnb = st["zsT"], st["dept"], st["attnb"]
            # ---- A2 = zs @ G0^T  (G0 = WhN @ WfZ host-folded) ----
            p_a = p_big.tile([128, H], F32, tag="p_big")
            for ns in (slice(0, 512), slice(512, H)):
                for kc in range(KC):
                    nc.tensor.matmul(p_a[:, ns], zsT[:, kc, :],
                                     g0T[:, kc, ns],
                                     start=(kc == 0), stop=(kc == KC - 1))
            a2b = spool.tile([128, H], BF16, tag="ab")
            nc.scalar.copy(a2b[:], p_a[:])

            # ---- temp partial: -bertS + zs@WhZ^T + attn@A2 (early) ----
            # temp(b) accumulates (temp - bertS) so the blend is one stt.
            p_t = p_big.tile([128, H], F32, tag="p_big")
            bertST_t = st["bertST"]
            for ns in (slice(0, 512), slice(512, H)):
                for kc in range(KC):
                    nc.tensor.matmul(p_t[:, ns], zsT[:, kc, :],
                                     whzT[:, kc, ns],
                                     start=(kc == 0), stop=False)
                nc.tensor.matmul(p_t[:, ns], st["attnT"][:], a2b[:, ns],
                                 start=False, stop=False)
            for hc in range(KC):
                ns = slice(hc * 128, (hc + 1) * 128)
                nc.tensor.matmul(p_t[:, ns], bertST_t[:, hc, :], id_negb[:],
                                 start=False, stop=False)

            # ---- D = reduce_j(attn * dep); Pool share densified via ACT ----
            tmp2 = tpool.tile([128, L, E], BF16, tag="ttmp")
            nc.vector.tensor_tensor(
                tmp2[:, 0:64, :], dept[:, 0:64, :],
                attnb[:, 0:64].unsqueeze(2).broadcast_to([128, 64, E]),
                op=OP.mult)
            nc.vector.tensor_tensor(
                tmp2[:, 64:L, :], dept[:, 64:L, :],
                attnb[:, 64:L].unsqueeze(2).broadcast_to([128, 64, E]),
                op=OP.mult)
            # asymmetric pairwise tree: DVE folds j[0:96), Pool j[96:128)
            tmp3 = tpool.tile([128, L, E], BF16, tag="ttmp")
            t2, t3 = tmp2, tmp3
            nc.vector.tensor_tensor(t3[:, 64:96, :], t2[:, 64:96, :],
                                    t2[:, 96:128, :], op=OP.add)
            nc.vector.tensor_tensor(t2[:, 64:80, :], t3[:, 64:80, :],
                                    t3[:, 80:96, :], op=OP.add)
            nc.vector.tensor_tensor(t3[:, 64:72, :], t2[:, 64:72, :],
                                    t2[:, 72:80, :], op=OP.add)
            nc.vector.tensor_tensor(t2[:, 64:68, :], t3[:, 64:68, :],
                                    t3[:, 68:72, :], op=OP.add)
            nc.vector.tensor_tensor(t3[:, 64:66, :], t2[:, 64:66, :],
                                    t2[:, 66:68, :], op=OP.add)
            nc.vector.tensor_tensor(t2[:, 64:65, :], t3[:, 64:65, :],
                                    t3[:, 65:66, :], op=OP.add)
            nc.gpsimd.tensor_tensor(t3[:, 0:32, :], t2[:, 0:32, :],
                                    t2[:, 32:64, :], op=OP.add)
            nc.gpsimd.tensor_tensor(t2[:, 0:16, :], t3[:, 0:16, :],
                                    t3[:, 16:32, :], op=OP.add)
            nc.gpsimd.tensor_tensor(t3[:, 0:8, :], t2[:, 0:8, :],
                                    t2[:, 8:16, :], op=OP.add)
            nc.gpsimd.tensor_tensor(t2[:, 0:4, :], t3[:, 0:4, :],
                                    t3[:, 4:8, :], op=OP.add)
            nc.gpsimd.tensor_tensor(t3[:, 0:2, :], t2[:, 0:2, :],
                                    t2[:, 2:4, :], op=OP.add)
            nc.gpsimd.tensor_tensor(t2[:, 0:1, :], t3[:, 0:1, :],
                                    t3[:, 1:2, :], op=OP.add)
            dvb = spool.tile([128, E], BF16, tag="dvb")
            nc.vector.tensor_tensor(dvb[:], t2[:, 0, :], t2[:, 64, :],
                                    op=OP.add)
            if b == 0:
                dump("d_dvec", dvb[:])

            # D^T via PE transpose
            p_ad = st["p_ad"]
            nc.tensor.transpose(p_ad[0:E, 128:256], dvb[:], id_bf[:])
            dT = spool.tile([E, 128], BF16, tag="dT")
            nc.scalar.copy(dT[:], p_ad[0:E, 128:256])

            # ---- temp final: += D @ G1 (G1 = (WhN @ WfE)^T host-folded) ----
            for ns in (slice(0, 512), slice(512, H)):
                nc.tensor.matmul(p_t[:, ns], dT[:], g1[:, ns],
                                 start=False, stop=True)
            st["p_t"] = p_t

            # ---- upd mask column (host-folded span & any-neighbor) ----
            p_v = p_tr.tile([128, 512], F32, tag="p_tr")
            nc.tensor.matmul(p_v[:, 0:1], vrow4[0:1, b, :], ones_b[0:1, 0:1],
                             start=True, stop=True)
            # ---- blend: out = upd*(temp - bertS) + bertS; rolled store ----
            outt = opool.tile([128, H], F32, tag="outt")
            nc.vector.scalar_tensor_tensor(
                outt[:], p_t[:], p_v[:, 0:1], st["bertS"][:],
                op0=OP.mult, op1=OP.add)
            nc.sync.dma_start(out[b, 1:128, :], outt[0:127, :])
            nc.sync.dma_start(out[b, 0:1, :], outt[127:128, :])

        for b in range(PB):
            st = emit_front(b)
            emit_back(b, st)


def _get_nc():
    if "nc" not in _CACHED:
        _CACHED["nc"] = _build()
    return _CACHED["nc"]


def _chunkT(w):
    """W [rows, K] -> W^T chunk-major [128, K//128, rows] (lhsT layout)."""
    rows, k = w.shape
    return np.ascontiguousarray(
        w.T.reshape(k // 128, 128, rows).transpose(1, 0, 2))


def _prep_in_maps(bert_hidden_states, dep_type_adj, deprel_adj,
                  asp_start, asp_end, Wz, bz, wa, ba, Wf, Wh):
    bf = ml_dtypes.bfloat16
    bert = np.ascontiguousarray(np.asarray(bert_hidden_states, np.float32))
    wa_f0 = np.asarray(wa, np.float32)
    wae_f = wa_f0[2 * H:]
    wae_safe = np.where(wae_f == 0.0, 1.0, wae_f)
    dep = (np.asarray(dep_type_adj, np.float32)
           * wae_f[None, None, None, :]).astype(bf)
    adjf = np.ascontiguousarray(np.asarray(deprel_adj).astype(np.float32))
    # bertS^T chunk-major per batch: rows shifted by one (the z-roll)
    bs = np.ascontiguousarray(np.roll(bert, -1, axis=1))
    bertsT = np.ascontiguousarray(
        bs.transpose(0, 2, 1).reshape(B, KC, 128, L).transpose(0, 2, 1, 3)
    ).astype(bf)
    pos = np.arange(L, dtype=np.float32)
    s_ = np.asarray(asp_start).astype(np.float32)[:, None]
    e_ = np.asarray(asp_end).astype(np.float32)[:, None]
    vrow_full = (((pos[None, :] >= s_) & (pos[None, :] <= e_))
                 & (np.asarray(deprel_adj) > 0).any(-1)).astype(ml_dtypes.bfloat16)

    Wz = np.asarray(Wz, np.float32)
    Wf = np.asarray(Wf, np.float32)
    Wh = np.asarray(Wh, np.float32)
    wa_f = wa_f0
    wzT = _chunkT(Wz).astype(bf)
    g0T = _chunkT(Wh[:, :H] @ Wf[:, :H]).astype(bf)
    whzT = _chunkT(Wh[:, H:]).astype(bf)
    g1 = np.ascontiguousarray(
        (Wh[:, :H] @ Wf[:, H:]).T / wae_safe[:, None]).astype(bf)
    w2T = _chunkT(wa_f[:2 * H].reshape(2, H)).astype(bf)
    bzb = np.asarray(bz, np.float32)[None, :].astype(bf)
    bab = np.asarray(ba, np.float32).reshape(1, 1)

    in_maps = []
    for c in range(NCORES):
        s = slice(c * PB, (c + 1) * PB)
        in_maps.append(dict(
            berts=bs[s], bertsT=np.ascontiguousarray(bertsT[s]),
            dep=dep[s], adjf=adjf[s],
            vrow=np.ascontiguousarray(vrow_full[s][None, :, :]),
            wzT=wzT, g0T=g0T, whzT=whzT, g1=g1, w2T=w2T,
            bzt=bzb, bat=bab,
        ))
    return in_maps


def kernel(bert_hidden_states, dep_type_adj, deprel_adj, asp_start, asp_end,
           Wz, bz, wa, ba, Wf, Wh):
    from concourse.bass_utils import run_bass_kernel_spmd

    in_maps = _prep_in_maps(bert_hidden_states, dep_type_adj, deprel_adj,
                            asp_start, asp_end, Wz, bz, wa, ba, Wf, Wh)
    nc = _get_nc()
    res = run_bass_kernel_spmd(nc, in_maps, core_ids=list(range(NCORES)),
                               trace=bool(_CACHED.get("trace")),
                               tmpdir=_CACHED.get("trace_tmpdir"))
    _CACHED["last_results"] = res
    outs = [res.results[c]["out"] for c in range(NCORES)]
    return np.concatenate(outs, axis=0).astype(np.float32)

